# revision 1
# baseline (speedup 1.0000x reference)
"""Trainium2 Bass kernel for CustomConvolution2d.

Problem: y = conv2d(x, weight, stride=1, pad=1) + bias
  x: [32, 64, 128, 128] f32, weight: [64, 64, 3, 3] f32, bias: [64] f32.

Strategy (data-parallel, batch/8 = 4 images per core):

Per image, x is host-padded to [64, 130, 130] and loaded into SBUF
partitions 0-63; partitions 64-127 hold the same buffer shifted +1 row
(on-chip SBUF->SBUF copy). A matmul whose rhs spans partitions
(e, ci) = (row-shift, channel) then sees x rows r and r+1 at once, so
K = 128 is fully used.

The 3x3 conv over a block of `br` output rows (4-row groups of free
dim N = 512 per matmul / PSUM bank) is 3 matmuls per group (one per
kw), accumulating in PSUM, with lhsT

    [[W(kh=1,kw), W(kh=0,kw)],
     [W(kh=2,kw),     0     ]]   (K blocks = e, M blocks = d)

so PSUM partitions 0-63  (P0) get the kh=1,2 taps of rows r0+j, and
partitions 64-127 (P1) get the kh=0 tap of rows r0+j+1.  12 of the 16
lhsT quadrants are useful -> 75% PE utilization at K=M=128, N=512.
Weights stream at 1 cycle/row via float32r (N=512 >= 256).

Eviction per block: ACT writes Identity(P0 + bias) into the output
SBUF chunk (lane-locked, partitions 0-63); DVE then adds the shifted
P1 window in place (the DVE PSUM read port permits the partition-base
offset of 64).  The per-image row-shifted duplicate runs on the SWDGE
queue, stores alternate between the SP and SWDGE queues, and dummy
matmuls absorb DMA semaphore waits so real fp32r matmuls stay within
their single LW-struct wait slot.
"""

import numpy as np

N_FULL = 32
C = 64
H = 128
W = 128
HP = H + 2  # 130
NCORES = 8
NPER = N_FULL // NCORES  # 4 images per core

_cache = {}


def _build(dt_name: str, variant: str = "full", reps: int = 1,
           br: int = 8, loopn: int = 0):
    """Build the Bass program once per dtype. Returns the Bass object."""
    import concourse.bass as bass
    import concourse.tile as tile
    from concourse import bacc, mybir

    DT = getattr(mybir.dt, dt_name)
    F32 = mybir.dt.float32
    IDENT = mybir.ActivationFunctionType.Identity

    nc = bacc.Bacc(trn_type="TRN2", target_bir_lowering=False, debug=False,
                   num_devices=NCORES)

    xp = nc.dram_tensor("xp", [NPER, C, HP, HP], DT, kind="ExternalInput").ap()
    wpack = nc.dram_tensor("wpack", [3, 128, 128], DT, kind="ExternalInput").ap()
    biasb = nc.dram_tensor("biasb", [128, 1], F32, kind="ExternalInput").ap()
    out = nc.dram_tensor("out", [NPER, C, H, W], F32, kind="ExternalOutput").ap()

    nbanks = br // 4          # PSUM banks per accumulator slot
    nslot = 8 // nbanks       # number of slots (all 8 banks used)
    bw = 128 * br             # free-dim width of one slot
    nblk = H // br            # blocks per image
    bpc = 32 // br            # blocks per 32-row output chunk

    with tile.TileContext(nc) as tc:
        with (
            tc.tile_pool(name="wpool", bufs=1) as wpool,
            tc.tile_pool(name="cpool", bufs=1) as cpool,
            tc.tile_pool(name="xpool", bufs=2) as xpool,
            tc.tile_pool(name="opool", bufs=2) as opool,
            tc.tile_pool(name="ppool", bufs=1, space="PSUM") as ppool,
        ):
            wk = []
            for k in range(3):
                wt = wpool.tile([128, 128], DT, name=f"wk{k}")
                nc.sync.dma_start(out=wt[:, :], in_=wpack[k])
                wk.append(wt)
            # bias vector over all 128 partitions: rows 0-63 = bias(co),
            # rows 64-127 = 0 (the P1 half gets no bias)
            bias_sb = cpool.tile([128, 1], F32)
            nc.sync.dma_start(out=bias_sb[:, :], in_=biasb[:, :])

            # Two persistent 4-bank PSUM accumulators; 16 output rows each.
            # Separate tensors (not one big tile) so Tile's PSUM hazard
            # tracking never serializes a matmul against the other slot's
            # eviction reads.
            pa = [ppool.tile([128, bw], F32, name=f"pacc{i}")
                  for i in range(nslot)]

            # Dummy matmuls: let PE observe each weight-DMA semaphore here
            # (one lane per matmul) so real matmuls never wait on them.
            # fp32r requires full 128-column tiling and even innermost count,
            # so dummies are M=128, N=2.  They scribble on pa[0][:, 0:2],
            # which is safe: the next real accumulation group's start=True
            # clears the bank's has_written bits, overwriting the garbage.
            for k in range(3):
                nc.tensor.matmul(pa[0][:, 0:2], wk[k][:, :],
                                 wk[k][:, 0:2], start=True, stop=True)

            import contextlib
            loop_cm = (tc.For_i(0, loopn, 1) if loopn else
                       contextlib.nullcontext())
            with loop_cm:
                gb = 0  # global block counter
                for n in [i % NPER for i in range(reps * NPER)]:
                    x2 = xpool.tile([128, HP, HP], DT, name="x2")
                    # load + shifted dup in row chunks: Tile's subtile dep
                    # tracking lets dup chunk i start as soon as load chunks
                    # i/i+1 land, and lets early matmuls start before the
                    # whole image is resident.
                    lrows = [(0, 33), (33, 66), (66, 99), (99, HP)]
                    for (a, bnd) in lrows:
                        nc.sync.dma_start(out=x2[0:64, a:bnd, :],
                                          in_=xp[n, :, a:bnd, :])
                    # partitions 64-127 = same image shifted +1 row
                    # (SWDGE queue, so it does not serialize behind SP loads)
                    drows = [(0, 33), (33, 66), (66, 99), (99, HP - 1)]
                    for (a, bnd) in drows:
                        nc.gpsimd.dma_start(out=x2[64:128, a:bnd, :],
                                            in_=x2[0:64, a + 1:bnd + 1, :])
                    # dummy matmuls absorb the x-load + dup DMA waits for PE
                    # (one DMA semaphore each, keeping real matmuls at <=1 wait)
                    nc.tensor.matmul(pa[gb % nslot][:, 0:2], wk[0][0:64, :],
                                     x2[0:64, 0:1, 0:2], start=True, stop=True)
                    nc.tensor.matmul(pa[gb % nslot][:, 0:2], wk[0][:, :],
                                     x2[:, 0:1, 0:2], start=True, stop=True)

                    for c in range(4):  # output row chunks of 32
                        osb = opool.tile([C, 4096], F32, name="osb")
                        if variant in ("dmaOnly", "noEvict", "noDVE"):
                            # keep the tile "written" so Tile can allocate it
                            nc.vector.memset(osb[:, 0:8], 0.0)
                        for bc in range(bpc):  # blocks of br output rows
                            b = c * bpc + bc
                            r0 = br * b
                            s = gb % nslot
                            gb += 1
                            ps = pa[s]
                            psp = pa[(s - 1) % nslot]
                            if variant != "dmaOnly":
                                for g in range(nbanks):  # 4-row groups, 1 bank
                                    for k in range(3):
                                        nc.tensor.matmul(
                                            ps[:, g * 512:(g + 1) * 512],
                                            wk[k][:, :],
                                            x2[:, r0 + 4 * g + 1:r0 + 4 * g + 5,
                                               k:k + 128],
                                            start=(k == 0), stop=(k == 2))
                            if variant in ("dmaOnly", "noEvict"):
                                continue
                            o0 = bc * bw
                            # ACT evicts biased P0 straight into osb (lane-locked
                            # PSUM->SBUF, partitions 0-63), br rows per op.
                            nc.scalar.activation(
                                out=osb[:, o0:o0 + bw], in_=ps[0:64, :],
                                func=IDENT, bias=bias_sb[0:64, 0:1])
                            if variant == "noDVE":
                                continue
                            # DVE adds P1 in place; the PSUM read port permits the
                            # partition-base offset (64).
                            nc.vector.tensor_add(
                                osb[:, o0 + 128:o0 + bw],
                                osb[:, o0 + 128:o0 + bw],
                                ps[64:128, 0:bw - 128])
                            if b > 0:
                                # row r0 takes the previous block's P1 tail
                                nc.vector.tensor_add(
                                    osb[:, o0:o0 + 128],
                                    osb[:, o0:o0 + 128],
                                    psp[64:128, bw - 128:bw])
                        st_eng = nc.sync if c % 2 == 0 else nc.gpsimd
                        st_eng.dma_start(out=out[n, :, 32 * c:32 * c + 32, :],
                                         in_=osb[:, :])
    nc.compile()
    return nc


def _get_nc(dt_name: str, variant: str = "full", reps: int = 1, br: int = 8,
            loopn: int = 0):
    key = (dt_name, variant, reps, br, loopn)
    if key not in _cache:
        _cache[key] = _build(dt_name, variant, reps, br, loopn)
    return _cache[key]


_last_results = None


def prep_in_maps(x, weight, bias, dt_name="float32r"):
    """Host prep: pad x, pack lhsT weights, build per-core input maps."""
    x = np.ascontiguousarray(np.asarray(x), dtype=np.float32)
    weight = np.asarray(weight, dtype=np.float32)
    bias = np.asarray(bias, dtype=np.float32)

    if dt_name == "bfloat16":
        import ml_dtypes
        np_dt = ml_dtypes.bfloat16
    else:
        np_dt = np.float32

    # host prep: zero-pad x spatially
    xp = np.zeros((N_FULL, C, HP, HP), dtype=np_dt)
    xp[:, :, 1:HP - 1, 1:HP - 1] = x

    # lhsT pack: wpack[kw][e*64+ci, d*64+co]
    #   (e=0,d=0)=W[co,ci,1,kw]  (e=0,d=1)=W[co,ci,0,kw]
    #   (e=1,d=0)=W[co,ci,2,kw]  (e=1,d=1)=0
    wt = weight.transpose(1, 0, 2, 3)  # [ci, co, kh, kw]
    wpack = np.zeros((3, 128, 128), dtype=np_dt)
    for k in range(3):
        wpack[k, 0:64, 0:64] = wt[:, :, 1, k]
        wpack[k, 0:64, 64:128] = wt[:, :, 0, k]
        wpack[k, 64:128, 0:64] = wt[:, :, 2, k]

    biasb = np.zeros((128, 1), np.float32)
    biasb[0:C, 0] = bias

    nc = _get_nc(dt_name, "full", 1, 8)
    in_maps = [
        {"xp": xp[c * NPER:(c + 1) * NPER], "wpack": wpack, "biasb": biasb}
        for c in range(NCORES)
    ]
    return in_maps, nc


def kernel(x, weight, bias, dt_name="float32r", trace=False, br=8):
    global _last_results
    from concourse import bass_utils

    in_maps, nc = prep_in_maps(x, weight, bias, dt_name)
    nc = _get_nc(dt_name, "full", 1, br)
    res = bass_utils.run_bass_kernel_spmd(nc, in_maps, list(range(NCORES)),
                                          trace=trace)
    _last_results = res
    return np.concatenate([res.results[c]["out"] for c in range(NCORES)],
                          axis=0)

